# revision 36
# baseline (speedup 1.0000x reference)
"""Distributed causal multi-head attention layer for one TRN2 chip (8 NeuronCores).

Problem: S=2048, B=4, D=512, H=8 heads (DH=64), causal mask, fp32 I/O.

Sharding: core c handles batch b = c//2 and heads [4*(c%2), 4*(c%2)+4).
Each core computes its 4 heads' attention for its batch; the host
concatenates per-core outputs (no cross-core collectives needed).

Design (per-core); measured ~136us vs the 199-227us v1 baseline:
  - bf16 inputs/weights (host-cast): halves input DMA vs fp32.
  - Projections on PE in bf16 (fp32 PSUM), 512-col chunks; per-chunk
    bias copy on ScalarE. v is written as fp16 with the output bias bv
    FOLDED IN (out = sum_k w*(v+bv)/sum_k w = attn+bv), eliminating the
    epilogue bias add.
  - Scores per head-pair: K=64 bf16 matmuls on disjoint PE row halves
    (concurrent pairs), [k,q]-transposed tiles in fp32 PSUM, one k-tile
    per group x 4 PSUM bufs (2-iteration score lookahead; measured
    faster than 2-tile groups x 2 bufs despite double the exp
    instruction count).
  - Softmax exp SPLIT ACROSS TWO ENGINES (single-engine exp was the
    ~70us bottleneck):
      ScalarE path: DVE tri-mask add (-1e9, one staircase-AP op covering
        both slots of a diag group) then activation(Exp, scale=1/8) ->
        fp16 w.
      DVE path: ONE tensor_scalar op: i16 = s*(A/8) + B' (Schraudolph
        exp bit-trick: the int16 bitcast AS fp16 approximates exp(s/8),
        max rel err ~4%, mean ~0; softmax renormalizes most of it).
        Diagonal groups on this path post-multiply the masked staircase
        by a 0/1 tri instead of the tri-add.
    Assignment alternates engines WITHIN each pair iteration so both
    engines always run concurrently (pair-aware; measured much faster
    than occupancy-only balancing).
  - AV: out_aug[65, q] += v_aug.T @ w per k-tile (fp16 x fp16, row 64 =
    softmax denominator via the fp16 ones-column of v_aug). AV for group
    g is emitted av_lag=3 iterations behind its exp, so the in-order PE
    queue never parks waiting on an exp result (sc->exp->av chain
    latency off the critical path; measured -18us).
  - Epilogue (evict-then-normalize, deferred): each head's PSUM
    accumulator is copied to SBUF right after its last AV (frees the
    bank for the next pair, bufs=2), then den row -> [128,8] DMA ->
    one reciprocal -> DMA back -> gpsimd partition_broadcast -> one DVE
    multiply in SBUF -> DMA out. The whole epilogue of pair P is
    EMITTED during pair P+1's first iteration: ops gated on P's AVs
    never head-of-line-block the strict-FIFO exp engines.
  - Proj work is interleaved into the attention sweeps as fillers
    (matmuls in one slot, their PSUM->SBUF copies deferred to the next)
    so PE streams while ScalarE/DVE chew exp, and half-1 input DMA
    overlaps sweep-0/1 compute.
reps>0 wraps the body in a hardware For_i loop for on-device timing.
"""

import numpy as np

import concourse.bass as bass
import concourse.tile as tile
from concourse import bacc, mybir
from concourse.ap import AP
from concourse.bass_utils import run_bass_kernel_spmd

S, B, D, H = 2048, 4, 512, 8
DH = D // H            # 64
HPC = 4                # heads per core
NCORE = 8
SW = 512               # q sweep width
NSW = S // SW          # 4
KT = 128               # key tile (partition dim)
NEG = np.float32(-1e9)

# Schraudolph fp16 exp constants: i16 = round(x*1024/ln2 + 15360 - C)
# applied to s/8, folding the 1/8: scale = (1024/ln2)/8
SCH_A = float(1024.0 / np.log(2.0) / 8.0)   # 184.664
SCH_B = float(15360.0 - 58.0)               # bias, C=58 centers rel err

F32 = mybir.dt.float32
BF16 = mybir.dt.bfloat16
FP16 = mybir.dt.float16
I16 = mybir.dt.int16

NDC = D // 128  # 4 d-chunks


def _group_sizes(nkt: int, gmax: int):
    ng = -(-nkt // gmax)
    base, rem = nkt // ng, nkt % ng
    return [base + (1 if i < rem else 0) for i in range(ng)]


def _stair(ap, o0: int, nslot: int, width: int):
    """Staircase view over a [128, n, 512] tile AP: piece i covers columns
    [o0+i*128, o0+i*128+width) of slot i, i.e. flat free offsets o0 + i*640."""
    base = ap.copy()
    dims = list(base.ap)
    # dims: [[pstride, 128], [slot_stride, n], [istride, 512]] element units
    pdim = dims[0]
    istride = dims[-1][0]
    slot_stride = dims[-2][0]
    assert slot_stride == 512 * istride, (dims,)
    return AP(
        tensor=base.tensor,
        offset=base.offset + o0 * istride,
        ap=[pdim, [slot_stride + KT * istride, nslot], [istride, width]],
    )


def build_nc(causal: bool, reps: int = 0, scope: str = "all", cfg: dict | None = None):
    """reps>0 wraps the body in a hardware loop (for on-device timing).

    scope: "all" (default) loops the whole body; "attn" runs DMA+proj once
    and loops only the attention phase; "proj" loops DMA+proj only.
    """
    cfg = dict(cfg or {})
    # engine assignment: pair-aware alternation -- within each pair iteration
    # head A's exp goes to one engine and head B's to the other, so the two
    # exps always run concurrently; every 3rd diagonal group is forced onto
    # the exact ScalarE path (accuracy headroom at equal measured speed)
    assign = cfg.get("assign", lambda kind, u, sw, gi:
                     (u + gi) % 2 == 0 and not (kind == "diag" and gi % 3 == 2))
    qkbias_scalar = cfg.get("qkbias_scalar", True)
    av_lag = cfg.get("av_lag", 3)
    gmax = cfg.get("gmax", 1)          # k-tiles per score group (PSUM banks)
    sc_bufs = cfg.get("sc_bufs", 4 // gmax)
    fast_start = cfg.get("fast_start", False)

    nc = bacc.Bacc("TRN2", target_bir_lowering=False, debug=False, num_devices=NCORE)

    xT = nc.declare_dram_parameter("xT", [D, S], BF16, isOutput=False)
    kxT = nc.declare_dram_parameter("kxT", [D, S], BF16, isOutput=False)
    vxT = nc.declare_dram_parameter("vxT", [D, S], BF16, isOutput=False)
    wv = nc.declare_dram_parameter("wv", [D, HPC * DH], BF16, isOutput=False)
    wqk = nc.declare_dram_parameter("wqk", [2, D, HPC * DH], BF16, isOutput=False)
    # constants blob [128, 776] f32:
    #  [0:256)   trineg x2 slots (upper-tri -1e9, else 0) for staircase add
    #  [256:512) tri01 x2 slots (upper-tri 1.0, else 0.0) for staircase mult
    #  [512:514) bqT  [514:516) bkT   [516:772) bv broadcast [HPC*DH]
    cst = nc.declare_dram_parameter("cst", [128, 772], F32, isOutput=False)
    out = nc.declare_dram_parameter("out", [HPC, DH, S], F32, isOutput=True)

    from contextlib import ExitStack
    with tile.TileContext(nc) as tc, ExitStack() as _st:
        persist = _st.enter_context(tc.tile_pool(name="persist", bufs=1))
        wpool = _st.enter_context(tc.tile_pool(name="wtile", bufs=6))
        rpool = _st.enter_context(tc.tile_pool(name="res", bufs=3))
        eppool = _st.enter_context(tc.tile_pool(name="eptmp", bufs=2))
        ps_sc = _st.enter_context(tc.tile_pool(name="ps_sc", bufs=sc_bufs, space="PSUM"))
        ps_pj = _st.enter_context(tc.tile_pool(name="ps_pj", bufs=2, space="PSUM"))
        ps_out = _st.enter_context(tc.tile_pool(name="ps_out", bufs=2, space="PSUM"))

        def enter_loop():
            if reps:
                _st.enter_context(tc.For_i(0, reps, 1))

        if scope != "attn":
            enter_loop()
        if True:
            # ---- constants + weights ----
            cst_sb = persist.tile([128, 772], F32, tag="cst")
            nc.scalar.dma_start(out=cst_sb[:], in_=cst[:])
            trineg2 = cst_sb[:, 0:256]     # [128, 2*128] viewed per use
            tri012 = cst_sb[:, 256:512]
            bq_sb = cst_sb[:, 512:514]
            bk_sb = cst_sb[:, 514:516]
            bv_sb = cst_sb[:, 516:772]     # [128, 256] broadcast bv

            wqk_sb = persist.tile([128, 2, NDC, HPC * DH], BF16, tag="wqk")
            nc.sync.dma_start(
                out=wqk_sb[:], in_=wqk.rearrange("t (dc p) j -> p t dc j", p=128))
            wq_sb = wqk_sb[:, 0]
            wk_sb = wqk_sb[:, 1]
            wv_sb = persist.tile([128, NDC, HPC * DH], BF16, tag="wv")
            nc.sync.dma_start(
                out=wv_sb[:], in_=wv.rearrange("(dc p) j -> p dc j", p=128))

            x_sb = persist.tile([128, NDC, S], BF16, tag="x")
            kx_sb = persist.tile([128, NDC, S], BF16, tag="kx")
            qT_sb = persist.tile([128, 2, S], BF16, tag="qT")
            kTz_sb = persist.tile([128, HPC, S], BF16, tag="kTz")
            v_sb = persist.tile([128, S // 128, HPC, DH + 1], FP16, tag="v")

            vxpool = _st.enter_context(tc.tile_pool(name="vxp", bufs=2))
            _vq = {}

            def in_dma(which, c):
                """DMA one 512-seq chunk (c in 0..3) of xT/kxT, or vx quarter."""
                ch = slice(c * 512, (c + 1) * 512)
                if which == "x":
                    nc.sync.dma_start(
                        out=x_sb[:, :, ch],
                        in_=xT.rearrange("(dc p) s -> p dc s", p=128)[:, :, ch])
                elif which == "kx":
                    nc.sync.dma_start(
                        out=kx_sb[:, :, ch],
                        in_=kxT.rearrange("(dc p) s -> p dc s", p=128)[:, :, ch])
                else:  # vx
                    vq = vxpool.tile([128, NDC, 512], BF16, tag="vxs")
                    _vq[c] = vq
                    nc.sync.dma_start(
                        out=vq[:],
                        in_=vxT.rearrange("(dc p) s -> p dc s", p=128)[:, :, ch])

            def proj_chunk(qk: int, g: int, c: int):
                """Project one 512-seq chunk of q (qk=0) or k (qk=1) for
                head-group g (128 out dims = 2 heads)."""
                w_sb, b_sb, src = (
                    (wq_sb, bq_sb, x_sb) if qk == 0 else (wk_sb, bk_sb, kx_sb))
                ch = slice(c * 512, (c + 1) * 512)
                ps = ps_pj.tile([128, 512], F32, tag="pj")
                for dc in range(NDC):
                    nc.tensor.matmul(
                        ps[:, 0:512],
                        w_sb[:, dc, g * 128:(g + 1) * 128],
                        src[:, dc, ch],
                        start=(dc == 0),
                        stop=(dc == NDC - 1),
                    )
                def bias_copy():
                    if qk == 1:
                        # k rows land in the same row range as the head's q rows
                        for ho in range(2):
                            rs = slice(ho * DH, (ho + 1) * DH)
                            if qkbias_scalar:
                                nc.scalar.activation(
                                    out=kTz_sb[rs, 2 * g + ho, ch],
                                    in_=ps[rs, 0:512],
                                    func=mybir.ActivationFunctionType.Identity,
                                    bias=b_sb[rs, g:g + 1],
                                )
                            else:
                                nc.vector.tensor_scalar_add(
                                    out=kTz_sb[rs, 2 * g + ho, ch],
                                    in0=ps[rs, 0:512],
                                    scalar1=b_sb[rs, g:g + 1],
                                )
                    else:
                        if qkbias_scalar:
                            nc.scalar.activation(
                                out=qT_sb[:, g, ch], in_=ps[:, 0:512],
                                func=mybir.ActivationFunctionType.Identity,
                                bias=b_sb[:, g:g + 1],
                            )
                        else:
                            nc.vector.tensor_scalar_add(
                                out=qT_sb[:, g, ch], in0=ps[:, 0:512],
                                scalar1=b_sb[:, g:g + 1],
                            )
                return bias_copy

            def v_chunk(st: int):
                """Project one 128-seq tile of v (all 4 heads), fold bv."""
                qi = st // 4
                vq = _vq[qi]
                st4 = st % 4
                ps = ps_pj.tile([128, 512], F32, tag="pj")
                for dc in range(NDC):
                    nc.tensor.matmul(
                        ps[:, 0:HPC * DH],
                        vq[:, dc, st4 * 128:(st4 + 1) * 128],
                        wv_sb[:, dc, :],
                        start=(dc == 0),
                        stop=(dc == NDC - 1),
                    )
                def v_copy():
                    # v = proj + bv (broadcast along seq partitions), fp16 out
                    nc.vector.tensor_tensor(
                        out=v_sb[:, st, :, 0:DH],
                        in0=ps[:, 0:HPC * DH].rearrange("p (u d) -> p u d", u=HPC),
                        in1=bv_sb[:].rearrange("p (u d) -> p u d", u=HPC),
                        op=mybir.AluOpType.add,
                    )
                return v_copy

            # ---------------- attention ----------------
            def _unit(u, sw):
                g = u // 2
                hp = slice(0, DH) if u % 2 == 0 else slice(DH, 128)
                qh = qT_sb[hp, g, :]      # [64, S]
                kh = kTz_sb[hp, u, :]     # [64, S]
                q0 = sw * SW
                nkt = (q0 + SW) // KT if causal else S // KT

                groups, k0 = [], 0
                for gs in _group_sizes(nkt, gmax):
                    groups.append(list(range(k0, k0 + gs)))
                    k0 += gs

                def off(kt):
                    return max(0, kt * KT - q0) if causal else 0

                st = {"o_ps": None, "gidx": {"full": 0, "diag": 0}}

                def emit_scores(kts):
                    n = len(kts)
                    grp = ps_sc.tile([128, n, 512], F32, tag="sc")
                    for slot, kt in enumerate(kts):
                        o = off(kt)
                        nc.tensor.matmul(
                            grp[:, slot, o:SW],
                            kh[:, kt * KT:(kt + 1) * KT],
                            qh[:, q0 + o:q0 + SW],
                            start=True, stop=True,
                        )
                    diag = causal and kts[-1] * KT >= q0
                    o0 = off(kts[0])
                    kind = "diag" if diag else "full"
                    idx = st["gidx"][kind]; st["gidx"][kind] += 1
                    use_dve = assign(kind, u, sw, idx)

                    # skip the dead sub-offset columns [0, o0) of slot 0:
                    # exp the contiguous flat range [o0, n*512)
                    nflat = n * 512
                    if not use_dve:
                        if diag:
                            # staircase tri-add (-1e9) over the diagonal
                            # 128-col block of each slot, one DVE op
                            nc.vector.tensor_tensor(
                                out=_stair(grp[:], o0, n, KT),
                                in0=_stair(grp[:], o0, n, KT),
                                in1=trineg2.rearrange(
                                    "p (t q) -> p t q", t=2)[:, 0:n, :],
                                op=mybir.AluOpType.add,
                            )
                        w = wpool.tile([128, n, 512], FP16, tag="ws")
                        nc.scalar.activation(
                            out=w[:].rearrange("p n q -> p (n q)")[:, o0:nflat],
                            in_=grp[:].rearrange("p n q -> p (n q)")[:, o0:nflat],
                            func=mybir.ActivationFunctionType.Exp, scale=0.125,
                        )
                        return w
                    # DVE path: Schraudolph bit-trick exp in one op
                    w16 = wpool.tile([128, n, 512], I16, tag="wd")
                    nc.vector.tensor_scalar(
                        out=w16[:].rearrange("p n q -> p (n q)")[:, o0:nflat],
                        in0=grp[:].rearrange("p n q -> p (n q)")[:, o0:nflat],
                        scalar1=SCH_A, scalar2=SCH_B,
                        op0=mybir.AluOpType.mult, op1=mybir.AluOpType.add,
                    )
                    w = w16[:].bitcast(FP16)
                    if diag:
                        # zero the masked upper-tri of the diagonal blocks
                        nc.vector.tensor_tensor(
                            out=_stair(w, o0, n, KT),
                            in0=_stair(w, o0, n, KT),
                            in1=tri012.rearrange(
                                "p (t q) -> p t q", t=2)[:, 0:n, :],
                            op=mybir.AluOpType.mult,
                        )
                    return w

                def emit_av(kts, w):
                    if st["o_ps"] is None:
                        st["o_ps"] = ps_out.tile(
                            [DH + 1, 512], F32, tag="out", name=f"o_ps_u{u}_s{sw}")
                    for slot, kt in enumerate(kts):
                        o = off(kt)
                        nc.tensor.matmul(
                            st["o_ps"][:, o:SW],
                            v_sb[:, kt, u, :],
                            w[:, slot, o:SW],
                            start=(kt == 0),
                            stop=(kt == nkt - 1),
                        )

                def get_ops():
                    return st["o_ps"]

                return groups, emit_scores, emit_av, get_ops

            def pair_epilogue(u0, u1, sw, ops_a, ops_b):
                """Evict-then-normalize: copy each head's PSUM accumulator to
                SBUF right away (frees the bank), then run the denominator
                round-trip and normalize entirely from SBUF."""
                q0 = sw * SW
                osbs = []
                for slot, (u, o_ps) in enumerate(((u0, ops_a), (u1, ops_b))):
                    o_sb = rpool.tile([DH + 1, 512], F32, tag="osb",
                                      name=f"osb_u{u}_s{sw}")
                    if (u + sw) % 2 == 0:
                        nc.scalar.activation(
                            out=o_sb[:], in_=o_ps[:],
                            func=mybir.ActivationFunctionType.Copy)
                    else:
                        nc.vector.tensor_copy(out=o_sb[:], in_=o_ps[:])
                    osbs.append(o_sb)
                den8 = eppool.tile([128, 2, 4], F32, tag="den8",
                                   name=f"den8_p{u0}_s{sw}")
                for slot, o_sb in enumerate(osbs):
                    nc.gpsimd.dma_start(
                        out=den8[:, slot, :],
                        in_=o_sb[DH:DH + 1, :].rearrange("o (p j) -> o p j", p=128))
                nc.vector.reciprocal(
                    out=den8[:].rearrange("p t j -> p (t j)"),
                    in_=den8[:].rearrange("p t j -> p (t j)"))
                r01 = eppool.tile([1, 2, 512], F32, tag="r01",
                                  name=f"r01_p{u0}_s{sw}")
                for slot in range(2):
                    nc.gpsimd.dma_start(
                        out=r01[:, slot, :].rearrange("o (p j) -> o p j", p=128),
                        in_=den8[:, slot, :])
                for slot, (u, o_sb) in enumerate(((u0, osbs[0]), (u1, osbs[1]))):
                    db = eppool.tile([DH, 512], F32, tag="db",
                                     name=f"db_u{u}_s{sw}")
                    nc.gpsimd.partition_broadcast(db[:], r01[:, slot, :])
                    nc.vector.tensor_tensor(
                        out=o_sb[0:DH, :], in0=o_sb[0:DH, :], in1=db[:],
                        op=mybir.AluOpType.mult)
                    nc.scalar.dma_start(
                        out=out[u, :, q0:q0 + SW], in_=o_sb[0:DH, :])

            late_q = []   # deferred post-matmul ops (bias/v copies)

            def pop_filler(fillers):
                """Emit one deferred copy, then one filler's matmuls (its
                copy is deferred to the next slot). Keeps AV/proj-gated
                vector ops from head-of-line-blocking the exp engines."""
                if late_q:
                    late_q.pop(0)()
                if fillers:
                    dl, fn = fillers.pop(0)
                    cl = fn()
                    if cl is not None:
                        late_q.append(cl)

            def attn_pair(u0, u1, sw, fillers=None, pending_ep=None):
                """Interleave two heads' sweeps; pop a filler after each
                score-group pair to keep PE fed with proj work. The PREVIOUS
                pair's epilogue is emitted after this pair's first group so
                its PSUM-gated ops never stall the exp engines."""
                ga, sca, ava, opsa = _unit(u0, sw)
                gb, scb, avb, opsb = _unit(u1, sw)
                assert len(ga) == len(gb)
                # AV runs `av_lag` iterations behind its exp so the in-order
                # PE queue never parks waiting on an exp result
                pend = []
                for gi in range(len(ga)):
                    wa = sca(ga[gi])
                    wb = scb(gb[gi])
                    if len(pend) >= max(1, av_lag):
                        for (av, kts, w) in pend.pop(0):
                            av(kts, w)
                    if gi == 1 and pending_ep is not None:
                        pending_ep()
                        pending_ep = None
                    elif fillers is not None:
                        pop_filler(fillers)
                    pend.append([(ava, ga[gi], wa), (avb, gb[gi], wb)])
                for gen in pend:
                    for (av, kts, w) in gen:
                        av(kts, w)
                if pending_ep is not None:
                    pending_ep()
                return lambda: pair_epilogue(u0, u1, sw, opsa(), opsb())

            # v ones column (fp16)
            nc.vector.memset(v_sb[:, :, :, DH], 1.0)

            def dma_preamble():
                # order matters on the sync queue: earliest-needed first
                in_dma("x", 0); in_dma("kx", 0); in_dma("vx", 0)
                in_dma("x", 1); in_dma("kx", 1); in_dma("vx", 1)
                in_dma("x", 2); in_dma("kx", 2)
                in_dma("x", 3); in_dma("kx", 3)
                in_dma("vx", 2); in_dma("vx", 3)

            def proj_stage0():
                """Minimum before pair(0,1,0): its own q/k chunk 0 (head
                group 0 only when fast_start) and v st 0-3."""
                cls = []
                for g in range(1 if fast_start else 2):
                    cls.append(proj_chunk(0, g, 0))
                    cls.append(proj_chunk(1, g, 0))
                for t in range(4):
                    cls.append(v_chunk(t))
                for cl in cls:
                    cl()

            def make_fillers():
                """(deadline_pair, fn), pair index p = 2*sweep + pairpos.
                q/k chunk c of head-group g feeds pair 2c+g (pair (0,1) uses
                only group-0 projections, pair (2,3) only group-1); v st
                feeds pair 2*(st//4). Sorted by deadline."""
                f = []
                if fast_start:
                    f.append((1, lambda: proj_chunk(0, 1, 0)))
                    f.append((1, lambda: proj_chunk(1, 1, 0)))
                for c in range(1, 4):
                    for g in range(2):
                        f.append((2 * c + g, lambda g=g, c=c: proj_chunk(0, g, c)))
                        f.append((2 * c + g, lambda g=g, c=c: proj_chunk(1, g, c)))
                    for t in range(4 * c, 4 * c + 4):
                        f.append((2 * c, lambda t=t: v_chunk(t)))
                f.sort(key=lambda e: e[0])
                # pad with no-ops so attn_pair can always pop
                for _ in range(96):
                    f.append((99, lambda: None))
                return f

            def drain(fillers, upto):
                """Flush deferred copies and emit every filler due by
                pair-index `upto` (copies included, immediately)."""
                while late_q:
                    late_q.pop(0)()
                while fillers and fillers[0][0] <= upto:
                    dl, fn = fillers.pop(0)
                    cl = fn()
                    if cl is not None:
                        cl()

            def run_attn(fillers):
                ep = None
                for s in range(NSW):
                    for pp, (ua, ub) in enumerate(((0, 1), (2, 3))):
                        if fillers is not None:
                            drain(fillers, 2 * s + pp)
                        ep = attn_pair(ua, ub, s, fillers, ep)
                ep()

            if scope != "all":
                # timing probes: proj / attn phases separated
                dma_preamble()
                proj_stage0()
                fillers = make_fillers()
                drain(fillers, 98)
                if scope == "attn":
                    enter_loop()
                    run_attn(None)
            else:
                dma_preamble()
                proj_stage0()
                if causal:
                    run_attn(make_fillers())
                else:
                    fillers = make_fillers()
                    drain(fillers, 98)
                    run_attn(None)

    nc.finalize()
    return nc


_NC_CACHE = {}


def _get_nc(causal: bool):
    if causal not in _NC_CACHE:
        _NC_CACHE[causal] = build_nc(causal)
    return _NC_CACHE[causal]


def make_in_maps(input_tensor, keys_vector, values_vector, Wq, bq, Wk, bk, Wv, bv):
    import ml_dtypes
    bf16 = ml_dtypes.bfloat16
    # scores tiles are [k, q] (transposed): keep k <= q  ->  mask the strictly
    # lower triangle (key index p > query column j)
    keep = np.triu(np.ones((KT, KT), dtype=bool))  # j >= p kept
    trineg = np.where(keep, np.float32(0), NEG).astype(np.float32)
    tri01 = np.where(keep, np.float32(1), np.float32(0)).astype(np.float32)
    in_maps = []
    for c in range(NCORE):
        b, hg = c // 2, c % 2
        hs = slice(hg * HPC * DH, (hg + 1) * HPC * DH)
        cst = np.zeros((128, 772), np.float32)
        cst[:, 0:128] = trineg
        cst[:, 128:256] = trineg
        cst[:, 256:384] = tri01
        cst[:, 384:512] = tri01
        cst[:, 512:514] = np.asarray(bq)[hs].reshape(2, 128).T
        cst[:, 514:516] = np.asarray(bk)[hs].reshape(2, 128).T
        cst[:, 516:772] = np.asarray(bv)[hs][None, :]
        m = {
            "xT": np.ascontiguousarray(
                np.asarray(input_tensor)[:, b, :].T).astype(bf16),
            "kxT": np.ascontiguousarray(
                np.asarray(keys_vector)[:, b, :].T).astype(bf16),
            "vxT": np.ascontiguousarray(
                np.asarray(values_vector)[:, b, :].T).astype(bf16),
            "wv": np.ascontiguousarray(np.asarray(Wv)[:, hs]).astype(bf16),
            "wqk": np.ascontiguousarray(
                np.stack([np.asarray(Wq)[:, hs], np.asarray(Wk)[:, hs]])
            ).astype(bf16),
            "cst": cst,
        }
        in_maps.append(m)
    return in_maps


def assemble_output(results):
    full = np.empty((S, B, D), dtype=np.float32)
    for c in range(NCORE):
        b, hg = c // 2, c % 2
        o = results[c]["out"]  # [HPC, DH, S]
        for u in range(HPC):
            h = hg * HPC + u
            full[:, b, h * DH:(h + 1) * DH] = o[u].T
    return full


def kernel(input_tensor, keys_vector, values_vector, Wq, bq, Wk, bk, Wv, bv, mask):
    causal = bool(np.asarray(mask).item()) if np.asarray(mask).size == 1 else True
    nc = _get_nc(causal)
    in_maps = make_in_maps(
        input_tensor, keys_vector, values_vector, Wq, bq, Wk, bk, Wv, bv
    )
    res = run_bass_kernel_spmd(nc, in_maps, core_ids=list(range(NCORE)))
    return assemble_output(res.results)


# revision 37
# speedup vs baseline: 1.0197x; 1.0197x over previous
"""Distributed causal multi-head attention layer for one TRN2 chip (8 NeuronCores).

Problem: S=2048, B=4, D=512, H=8 heads (DH=64), causal mask, fp32 I/O.

Sharding: core c handles batch b = c//2 and heads [4*(c%2), 4*(c%2)+4).
Each core computes its 4 heads' attention for its batch; the host
concatenates per-core outputs (no cross-core collectives needed).

Design (per-core); measured ~136us vs the 199-227us v1 baseline:
  - bf16 inputs/weights (host-cast): halves input DMA vs fp32.
  - Projections on PE in bf16 (fp32 PSUM), 512-col chunks; per-chunk
    bias copy on ScalarE. v is written as fp16 with the output bias bv
    FOLDED IN (out = sum_k w*(v+bv)/sum_k w = attn+bv), eliminating the
    epilogue bias add.
  - Scores per head-pair: K=64 bf16 matmuls on disjoint PE row halves
    (concurrent pairs), [k,q]-transposed tiles in fp32 PSUM, one k-tile
    per group x 4 PSUM bufs (2-iteration score lookahead; measured
    faster than 2-tile groups x 2 bufs despite double the exp
    instruction count).
  - Softmax exp SPLIT ACROSS TWO ENGINES (single-engine exp was the
    ~70us bottleneck):
      ScalarE path: DVE tri-mask add (-1e9, one staircase-AP op covering
        both slots of a diag group) then activation(Exp, scale=1/8) ->
        fp16 w.
      DVE path: ONE tensor_scalar op: i16 = s*(A/8) + B' (Schraudolph
        exp bit-trick: the int16 bitcast AS fp16 approximates exp(s/8),
        max rel err ~4%, mean ~0; softmax renormalizes most of it).
        Diagonal groups on this path post-multiply the masked staircase
        by a 0/1 tri instead of the tri-add.
    Assignment alternates engines WITHIN each pair iteration so both
    engines always run concurrently (pair-aware; measured much faster
    than occupancy-only balancing).
  - AV: out_aug[65, q] += v_aug.T @ w per k-tile (fp16 x fp16, row 64 =
    softmax denominator via the fp16 ones-column of v_aug). AV for group
    g is emitted av_lag=3 iterations behind its exp, so the in-order PE
    queue never parks waiting on an exp result (sc->exp->av chain
    latency off the critical path; measured -18us).
  - Epilogue (evict-then-normalize, deferred): each head's PSUM
    accumulator is copied to SBUF right after its last AV (frees the
    bank for the next pair, bufs=2), then den row -> [128,8] DMA ->
    one reciprocal -> DMA back -> gpsimd partition_broadcast -> one DVE
    multiply in SBUF -> DMA out. The whole epilogue of pair P is
    EMITTED during pair P+1's first iteration: ops gated on P's AVs
    never head-of-line-block the strict-FIFO exp engines.
  - Proj work is interleaved into the attention sweeps as fillers
    (matmuls in one slot, their PSUM->SBUF copies deferred to the next)
    so PE streams while ScalarE/DVE chew exp, and half-1 input DMA
    overlaps sweep-0/1 compute.
reps>0 wraps the body in a hardware For_i loop for on-device timing.
"""

import numpy as np

import concourse.bass as bass
import concourse.tile as tile
from concourse import bacc, mybir
from concourse.ap import AP
from concourse.bass_utils import run_bass_kernel_spmd

S, B, D, H = 2048, 4, 512, 8
DH = D // H            # 64
HPC = 4                # heads per core
NCORE = 8
SW = 512               # q sweep width
NSW = S // SW          # 4
KT = 128               # key tile (partition dim)
NEG = np.float32(-1e9)

# Schraudolph fp16 exp constants: i16 = round(x*1024/ln2 + 15360 - C)
# applied to s/8, folding the 1/8: scale = (1024/ln2)/8
SCH_A = float(1024.0 / np.log(2.0) / 8.0)   # 184.664
SCH_B = float(15360.0 - 58.0)               # bias, C=58 centers rel err

F32 = mybir.dt.float32
BF16 = mybir.dt.bfloat16
FP16 = mybir.dt.float16
I16 = mybir.dt.int16

NDC = D // 128  # 4 d-chunks


def _group_sizes(nkt: int, gmax: int):
    ng = -(-nkt // gmax)
    base, rem = nkt // ng, nkt % ng
    return [base + (1 if i < rem else 0) for i in range(ng)]


def _stair(ap, o0: int, nslot: int, width: int):
    """Staircase view over a [128, n, 512] tile AP: piece i covers columns
    [o0+i*128, o0+i*128+width) of slot i, i.e. flat free offsets o0 + i*640."""
    base = ap.copy()
    dims = list(base.ap)
    # dims: [[pstride, 128], [slot_stride, n], [istride, 512]] element units
    pdim = dims[0]
    istride = dims[-1][0]
    slot_stride = dims[-2][0]
    assert slot_stride == 512 * istride, (dims,)
    return AP(
        tensor=base.tensor,
        offset=base.offset + o0 * istride,
        ap=[pdim, [slot_stride + KT * istride, nslot], [istride, width]],
    )


def build_nc(causal: bool, reps: int = 0, scope: str = "all", cfg: dict | None = None):
    """reps>0 wraps the body in a hardware loop (for on-device timing).

    scope: "all" (default) loops the whole body; "attn" runs DMA+proj once
    and loops only the attention phase; "proj" loops DMA+proj only.
    """
    cfg = dict(cfg or {})
    # engine assignment: pair-aware alternation -- within each pair iteration
    # head A's exp goes to one engine and head B's to the other, so the two
    # exps always run concurrently; every 3rd diagonal group is forced onto
    # the exact ScalarE path (accuracy headroom at equal measured speed)
    assign = cfg.get("assign", lambda kind, u, sw, gi:
                     (u + gi) % 2 == 0 and not (kind == "diag" and gi % 3 == 2))
    qkbias_scalar = cfg.get("qkbias_scalar", True)
    qkbias_mode = cfg.get("qkbias_mode", "scal" if qkbias_scalar else "dve")
    evict_mode = cfg.get("evict_mode", "alt")
    av_lag = cfg.get("av_lag", 3)
    gmax = cfg.get("gmax", 1)          # k-tiles per score group (PSUM banks)
    sc_bufs = cfg.get("sc_bufs", 4 // gmax)
    fast_start = cfg.get("fast_start", False)

    nc = bacc.Bacc("TRN2", target_bir_lowering=False, debug=False, num_devices=NCORE)

    xT = nc.declare_dram_parameter("xT", [D, S], BF16, isOutput=False)
    kxT = nc.declare_dram_parameter("kxT", [D, S], BF16, isOutput=False)
    vxT = nc.declare_dram_parameter("vxT", [D, S], BF16, isOutput=False)
    wv = nc.declare_dram_parameter("wv", [D, HPC * DH], BF16, isOutput=False)
    wqk = nc.declare_dram_parameter("wqk", [2, D, HPC * DH], BF16, isOutput=False)
    # constants blob [128, 776] f32:
    #  [0:256)   trineg x2 slots (upper-tri -1e9, else 0) for staircase add
    #  [256:512) tri01 x2 slots (upper-tri 1.0, else 0.0) for staircase mult
    #  [512:514) bqT  [514:516) bkT   [516:772) bv broadcast [HPC*DH]
    cst = nc.declare_dram_parameter("cst", [128, 772], F32, isOutput=False)
    out = nc.declare_dram_parameter("out", [HPC, DH, S], F32, isOutput=True)

    from contextlib import ExitStack
    with tile.TileContext(nc) as tc, ExitStack() as _st:
        persist = _st.enter_context(tc.tile_pool(name="persist", bufs=1))
        wpool = _st.enter_context(tc.tile_pool(name="wtile", bufs=6))
        rpool = _st.enter_context(tc.tile_pool(name="res", bufs=3))
        eppool = _st.enter_context(tc.tile_pool(name="eptmp", bufs=2))
        ps_sc = _st.enter_context(tc.tile_pool(name="ps_sc", bufs=sc_bufs, space="PSUM"))
        ps_pj = _st.enter_context(tc.tile_pool(name="ps_pj", bufs=2, space="PSUM"))
        ps_out = _st.enter_context(tc.tile_pool(name="ps_out", bufs=2, space="PSUM"))

        def enter_loop():
            if reps:
                _st.enter_context(tc.For_i(0, reps, 1))

        if scope != "attn":
            enter_loop()
        if True:
            # ---- constants + weights ----
            cst_sb = persist.tile([128, 772], F32, tag="cst")
            nc.scalar.dma_start(out=cst_sb[:], in_=cst[:])
            trineg2 = cst_sb[:, 0:256]     # [128, 2*128] viewed per use
            tri012 = cst_sb[:, 256:512]
            bq_sb = cst_sb[:, 512:514]
            bk_sb = cst_sb[:, 514:516]
            bv_sb = cst_sb[:, 516:772]     # [128, 256] broadcast bv

            wqk_sb = persist.tile([128, 2, NDC, HPC * DH], BF16, tag="wqk")
            nc.sync.dma_start(
                out=wqk_sb[:], in_=wqk.rearrange("t (dc p) j -> p t dc j", p=128))
            wq_sb = wqk_sb[:, 0]
            wk_sb = wqk_sb[:, 1]
            wv_sb = persist.tile([128, NDC, HPC * DH], BF16, tag="wv")
            nc.sync.dma_start(
                out=wv_sb[:], in_=wv.rearrange("(dc p) j -> p dc j", p=128))

            x_sb = persist.tile([128, NDC, S], BF16, tag="x")
            kx_sb = persist.tile([128, NDC, S], BF16, tag="kx")
            qT_sb = persist.tile([128, 2, S], BF16, tag="qT")
            kTz_sb = persist.tile([128, HPC, S], BF16, tag="kTz")
            v_sb = persist.tile([128, S // 128, HPC, DH + 1], FP16, tag="v")

            vxpool = _st.enter_context(tc.tile_pool(name="vxp", bufs=2))
            _vq = {}

            def in_dma(which, c):
                """DMA one 512-seq chunk (c in 0..3) of xT/kxT, or vx quarter."""
                ch = slice(c * 512, (c + 1) * 512)
                if which == "x":
                    nc.sync.dma_start(
                        out=x_sb[:, :, ch],
                        in_=xT.rearrange("(dc p) s -> p dc s", p=128)[:, :, ch])
                elif which == "kx":
                    nc.sync.dma_start(
                        out=kx_sb[:, :, ch],
                        in_=kxT.rearrange("(dc p) s -> p dc s", p=128)[:, :, ch])
                else:  # vx
                    vq = vxpool.tile([128, NDC, 512], BF16, tag="vxs")
                    _vq[c] = vq
                    nc.sync.dma_start(
                        out=vq[:],
                        in_=vxT.rearrange("(dc p) s -> p dc s", p=128)[:, :, ch])

            def proj_chunk(qk: int, g: int, c: int):
                """Project one 512-seq chunk of q (qk=0) or k (qk=1) for
                head-group g (128 out dims = 2 heads)."""
                w_sb, b_sb, src = (
                    (wq_sb, bq_sb, x_sb) if qk == 0 else (wk_sb, bk_sb, kx_sb))
                ch = slice(c * 512, (c + 1) * 512)
                ps = ps_pj.tile([128, 512], F32, tag="pj")
                for dc in range(NDC):
                    nc.tensor.matmul(
                        ps[:, 0:512],
                        w_sb[:, dc, g * 128:(g + 1) * 128],
                        src[:, dc, ch],
                        start=(dc == 0),
                        stop=(dc == NDC - 1),
                    )
                def bias_copy():
                    use_scal = (qkbias_mode == "scal" or
                                (qkbias_mode == "alt" and (qk + g + c) % 2 == 0))
                    if qk == 1:
                        # k rows land in the same row range as the head's q rows
                        for ho in range(2):
                            rs = slice(ho * DH, (ho + 1) * DH)
                            if use_scal:
                                nc.scalar.activation(
                                    out=kTz_sb[rs, 2 * g + ho, ch],
                                    in_=ps[rs, 0:512],
                                    func=mybir.ActivationFunctionType.Identity,
                                    bias=b_sb[rs, g:g + 1],
                                )
                            else:
                                nc.vector.tensor_scalar_add(
                                    out=kTz_sb[rs, 2 * g + ho, ch],
                                    in0=ps[rs, 0:512],
                                    scalar1=b_sb[rs, g:g + 1],
                                )
                    else:
                        if use_scal:
                            nc.scalar.activation(
                                out=qT_sb[:, g, ch], in_=ps[:, 0:512],
                                func=mybir.ActivationFunctionType.Identity,
                                bias=b_sb[:, g:g + 1],
                            )
                        else:
                            nc.vector.tensor_scalar_add(
                                out=qT_sb[:, g, ch], in0=ps[:, 0:512],
                                scalar1=b_sb[:, g:g + 1],
                            )
                return bias_copy

            def v_chunk(st: int):
                """Project one 128-seq tile of v (all 4 heads), fold bv."""
                qi = st // 4
                vq = _vq[qi]
                st4 = st % 4
                ps = ps_pj.tile([128, 512], F32, tag="pj")
                for dc in range(NDC):
                    nc.tensor.matmul(
                        ps[:, 0:HPC * DH],
                        vq[:, dc, st4 * 128:(st4 + 1) * 128],
                        wv_sb[:, dc, :],
                        start=(dc == 0),
                        stop=(dc == NDC - 1),
                    )
                def v_copy():
                    # v = proj + bv (broadcast along seq partitions), fp16 out
                    nc.vector.tensor_tensor(
                        out=v_sb[:, st, :, 0:DH],
                        in0=ps[:, 0:HPC * DH].rearrange("p (u d) -> p u d", u=HPC),
                        in1=bv_sb[:].rearrange("p (u d) -> p u d", u=HPC),
                        op=mybir.AluOpType.add,
                    )
                return v_copy

            # ---------------- attention ----------------
            def _unit(u, sw):
                g = u // 2
                hp = slice(0, DH) if u % 2 == 0 else slice(DH, 128)
                qh = qT_sb[hp, g, :]      # [64, S]
                kh = kTz_sb[hp, u, :]     # [64, S]
                q0 = sw * SW
                nkt = (q0 + SW) // KT if causal else S // KT

                groups, k0 = [], 0
                for gs in _group_sizes(nkt, gmax):
                    groups.append(list(range(k0, k0 + gs)))
                    k0 += gs

                def off(kt):
                    return max(0, kt * KT - q0) if causal else 0

                st = {"o_ps": None, "gidx": {"full": 0, "diag": 0}}

                def emit_scores(kts):
                    n = len(kts)
                    grp = ps_sc.tile([128, n, 512], F32, tag="sc")
                    for slot, kt in enumerate(kts):
                        o = off(kt)
                        nc.tensor.matmul(
                            grp[:, slot, o:SW],
                            kh[:, kt * KT:(kt + 1) * KT],
                            qh[:, q0 + o:q0 + SW],
                            start=True, stop=True,
                        )
                    diag = causal and kts[-1] * KT >= q0
                    o0 = off(kts[0])
                    kind = "diag" if diag else "full"
                    idx = st["gidx"][kind]; st["gidx"][kind] += 1
                    use_dve = assign(kind, u, sw, idx)

                    # skip the dead sub-offset columns [0, o0) of slot 0:
                    # exp the contiguous flat range [o0, n*512)
                    nflat = n * 512
                    if not use_dve:
                        if diag:
                            # staircase tri-add (-1e9) over the diagonal
                            # 128-col block of each slot, one DVE op
                            nc.vector.tensor_tensor(
                                out=_stair(grp[:], o0, n, KT),
                                in0=_stair(grp[:], o0, n, KT),
                                in1=trineg2.rearrange(
                                    "p (t q) -> p t q", t=2)[:, 0:n, :],
                                op=mybir.AluOpType.add,
                            )
                        w = wpool.tile([128, n, 512], FP16, tag="ws")
                        nc.scalar.activation(
                            out=w[:].rearrange("p n q -> p (n q)")[:, o0:nflat],
                            in_=grp[:].rearrange("p n q -> p (n q)")[:, o0:nflat],
                            func=mybir.ActivationFunctionType.Exp, scale=0.125,
                        )
                        return w
                    # DVE path: Schraudolph bit-trick exp in one op
                    w16 = wpool.tile([128, n, 512], I16, tag="wd")
                    nc.vector.tensor_scalar(
                        out=w16[:].rearrange("p n q -> p (n q)")[:, o0:nflat],
                        in0=grp[:].rearrange("p n q -> p (n q)")[:, o0:nflat],
                        scalar1=SCH_A, scalar2=SCH_B,
                        op0=mybir.AluOpType.mult, op1=mybir.AluOpType.add,
                    )
                    w = w16[:].bitcast(FP16)
                    if diag:
                        # zero the masked upper-tri of the diagonal blocks
                        nc.vector.tensor_tensor(
                            out=_stair(w, o0, n, KT),
                            in0=_stair(w, o0, n, KT),
                            in1=tri012.rearrange(
                                "p (t q) -> p t q", t=2)[:, 0:n, :],
                            op=mybir.AluOpType.mult,
                        )
                    return w

                def emit_av(kts, w):
                    if st["o_ps"] is None:
                        st["o_ps"] = ps_out.tile(
                            [DH + 1, 512], F32, tag="out", name=f"o_ps_u{u}_s{sw}")
                    for slot, kt in enumerate(kts):
                        o = off(kt)
                        nc.tensor.matmul(
                            st["o_ps"][:, o:SW],
                            v_sb[:, kt, u, :],
                            w[:, slot, o:SW],
                            start=(kt == 0),
                            stop=(kt == nkt - 1),
                        )

                def get_ops():
                    return st["o_ps"]

                return groups, emit_scores, emit_av, get_ops

            def pair_epilogue(u0, u1, sw, ops_a, ops_b):
                """Evict-then-normalize: copy each head's PSUM accumulator to
                SBUF right away (frees the bank), then run the denominator
                round-trip and normalize entirely from SBUF."""
                q0 = sw * SW
                osbs = []
                for slot, (u, o_ps) in enumerate(((u0, ops_a), (u1, ops_b))):
                    o_sb = rpool.tile([DH + 1, 512], F32, tag="osb",
                                      name=f"osb_u{u}_s{sw}")
                    ev_scal = (evict_mode == "scal" or
                               (evict_mode == "alt" and (u + sw) % 2 == 0))
                    if ev_scal:
                        nc.scalar.activation(
                            out=o_sb[:], in_=o_ps[:],
                            func=mybir.ActivationFunctionType.Copy)
                    else:
                        nc.vector.tensor_copy(out=o_sb[:], in_=o_ps[:])
                    osbs.append(o_sb)
                den8 = eppool.tile([128, 2, 4], F32, tag="den8",
                                   name=f"den8_p{u0}_s{sw}")
                for slot, o_sb in enumerate(osbs):
                    nc.gpsimd.dma_start(
                        out=den8[:, slot, :],
                        in_=o_sb[DH:DH + 1, :].rearrange("o (p j) -> o p j", p=128))
                nc.vector.reciprocal(
                    out=den8[:].rearrange("p t j -> p (t j)"),
                    in_=den8[:].rearrange("p t j -> p (t j)"))
                r01 = eppool.tile([1, 2, 512], F32, tag="r01",
                                  name=f"r01_p{u0}_s{sw}")
                for slot in range(2):
                    nc.gpsimd.dma_start(
                        out=r01[:, slot, :].rearrange("o (p j) -> o p j", p=128),
                        in_=den8[:, slot, :])
                for slot, (u, o_sb) in enumerate(((u0, osbs[0]), (u1, osbs[1]))):
                    db = eppool.tile([DH, 512], F32, tag="db",
                                     name=f"db_u{u}_s{sw}")
                    nc.gpsimd.partition_broadcast(db[:], r01[:, slot, :])
                    nc.vector.tensor_tensor(
                        out=o_sb[0:DH, :], in0=o_sb[0:DH, :], in1=db[:],
                        op=mybir.AluOpType.mult)
                    nc.scalar.dma_start(
                        out=out[u, :, q0:q0 + SW], in_=o_sb[0:DH, :])

            late_q = []   # deferred post-matmul ops (bias/v copies)

            def pop_filler(fillers):
                """Emit one deferred copy, then one filler's matmuls (its
                copy is deferred to the next slot). Keeps AV/proj-gated
                vector ops from head-of-line-blocking the exp engines."""
                if late_q:
                    late_q.pop(0)()
                if fillers:
                    dl, fn = fillers.pop(0)
                    cl = fn()
                    if cl is not None:
                        late_q.append(cl)

            def attn_pair(u0, u1, sw, fillers=None, pending_ep=None):
                """Interleave two heads' sweeps; pop a filler after each
                score-group pair to keep PE fed with proj work. The PREVIOUS
                pair's epilogue is emitted after this pair's first group so
                its PSUM-gated ops never stall the exp engines."""
                ga, sca, ava, opsa = _unit(u0, sw)
                gb, scb, avb, opsb = _unit(u1, sw)
                assert len(ga) == len(gb)
                # AV runs `av_lag` iterations behind its exp so the in-order
                # PE queue never parks waiting on an exp result
                pend = []
                for gi in range(len(ga)):
                    wa = sca(ga[gi])
                    wb = scb(gb[gi])
                    if len(pend) >= max(1, av_lag):
                        for (av, kts, w) in pend.pop(0):
                            av(kts, w)
                    if gi == 1 and pending_ep is not None:
                        pending_ep()
                        pending_ep = None
                    elif fillers is not None:
                        pop_filler(fillers)
                    pend.append([(ava, ga[gi], wa), (avb, gb[gi], wb)])
                for gen in pend:
                    for (av, kts, w) in gen:
                        av(kts, w)
                if pending_ep is not None:
                    pending_ep()
                return lambda: pair_epilogue(u0, u1, sw, opsa(), opsb())

            # v ones column (fp16)
            nc.vector.memset(v_sb[:, :, :, DH], 1.0)

            def dma_preamble():
                # order matters on the sync queue: earliest-needed first
                in_dma("x", 0); in_dma("kx", 0); in_dma("vx", 0)
                in_dma("x", 1); in_dma("kx", 1); in_dma("vx", 1)
                in_dma("x", 2); in_dma("kx", 2)
                in_dma("x", 3); in_dma("kx", 3)
                in_dma("vx", 2); in_dma("vx", 3)

            def proj_stage0():
                """Minimum before pair(0,1,0): its own q/k chunk 0 (head
                group 0 only when fast_start) and v st 0-3."""
                cls = []
                for g in range(1 if fast_start else 2):
                    cls.append(proj_chunk(0, g, 0))
                    cls.append(proj_chunk(1, g, 0))
                for t in range(4):
                    cls.append(v_chunk(t))
                for cl in cls:
                    cl()

            def make_fillers():
                """(deadline_pair, fn), pair index p = 2*sweep + pairpos.
                q/k chunk c of head-group g feeds pair 2c+g (pair (0,1) uses
                only group-0 projections, pair (2,3) only group-1); v st
                feeds pair 2*(st//4). Sorted by deadline."""
                f = []
                if fast_start:
                    f.append((1, lambda: proj_chunk(0, 1, 0)))
                    f.append((1, lambda: proj_chunk(1, 1, 0)))
                for c in range(1, 4):
                    for g in range(2):
                        f.append((2 * c + g, lambda g=g, c=c: proj_chunk(0, g, c)))
                        f.append((2 * c + g, lambda g=g, c=c: proj_chunk(1, g, c)))
                    for t in range(4 * c, 4 * c + 4):
                        f.append((2 * c, lambda t=t: v_chunk(t)))
                f.sort(key=lambda e: e[0])
                # pad with no-ops so attn_pair can always pop
                for _ in range(96):
                    f.append((99, lambda: None))
                return f

            def drain(fillers, upto):
                """Flush deferred copies and emit every filler due by
                pair-index `upto` (copies included, immediately)."""
                while late_q:
                    late_q.pop(0)()
                while fillers and fillers[0][0] <= upto:
                    dl, fn = fillers.pop(0)
                    cl = fn()
                    if cl is not None:
                        cl()

            def run_attn(fillers):
                ep = None
                for s in range(NSW):
                    for pp, (ua, ub) in enumerate(((0, 1), (2, 3))):
                        if fillers is not None:
                            drain(fillers, 2 * s + pp)
                        ep = attn_pair(ua, ub, s, fillers, ep)
                ep()

            if scope != "all":
                # timing probes: proj / attn phases separated
                dma_preamble()
                proj_stage0()
                fillers = make_fillers()
                drain(fillers, 98)
                if scope == "attn":
                    enter_loop()
                    run_attn(None)
            else:
                dma_preamble()
                proj_stage0()
                if causal:
                    run_attn(make_fillers())
                else:
                    fillers = make_fillers()
                    drain(fillers, 98)
                    run_attn(None)

    nc.finalize()
    return nc


_NC_CACHE = {}


def _get_nc(causal: bool):
    if causal not in _NC_CACHE:
        _NC_CACHE[causal] = build_nc(causal)
    return _NC_CACHE[causal]


def make_in_maps(input_tensor, keys_vector, values_vector, Wq, bq, Wk, bk, Wv, bv):
    import ml_dtypes
    bf16 = ml_dtypes.bfloat16
    # scores tiles are [k, q] (transposed): keep k <= q  ->  mask the strictly
    # lower triangle (key index p > query column j)
    keep = np.triu(np.ones((KT, KT), dtype=bool))  # j >= p kept
    trineg = np.where(keep, np.float32(0), NEG).astype(np.float32)
    tri01 = np.where(keep, np.float32(1), np.float32(0)).astype(np.float32)
    in_maps = []
    for c in range(NCORE):
        b, hg = c // 2, c % 2
        hs = slice(hg * HPC * DH, (hg + 1) * HPC * DH)
        cst = np.zeros((128, 772), np.float32)
        cst[:, 0:128] = trineg
        cst[:, 128:256] = trineg
        cst[:, 256:384] = tri01
        cst[:, 384:512] = tri01
        cst[:, 512:514] = np.asarray(bq)[hs].reshape(2, 128).T
        cst[:, 514:516] = np.asarray(bk)[hs].reshape(2, 128).T
        cst[:, 516:772] = np.asarray(bv)[hs][None, :]
        m = {
            "xT": np.ascontiguousarray(
                np.asarray(input_tensor)[:, b, :].T).astype(bf16),
            "kxT": np.ascontiguousarray(
                np.asarray(keys_vector)[:, b, :].T).astype(bf16),
            "vxT": np.ascontiguousarray(
                np.asarray(values_vector)[:, b, :].T).astype(bf16),
            "wv": np.ascontiguousarray(np.asarray(Wv)[:, hs]).astype(bf16),
            "wqk": np.ascontiguousarray(
                np.stack([np.asarray(Wq)[:, hs], np.asarray(Wk)[:, hs]])
            ).astype(bf16),
            "cst": cst,
        }
        in_maps.append(m)
    return in_maps


def assemble_output(results):
    full = np.empty((S, B, D), dtype=np.float32)
    for c in range(NCORE):
        b, hg = c // 2, c % 2
        o = results[c]["out"]  # [HPC, DH, S]
        for u in range(HPC):
            h = hg * HPC + u
            full[:, b, h * DH:(h + 1) * DH] = o[u].T
    return full


def kernel(input_tensor, keys_vector, values_vector, Wq, bq, Wk, bk, Wv, bv, mask):
    causal = bool(np.asarray(mask).item()) if np.asarray(mask).size == 1 else True
    nc = _get_nc(causal)
    in_maps = make_in_maps(
        input_tensor, keys_vector, values_vector, Wq, bq, Wk, bk, Wv, bv
    )
    res = run_bass_kernel_spmd(nc, in_maps, core_ids=list(range(NCORE)))
    return assemble_output(res.results)
